# revision 1
# baseline (speedup 1.0000x reference)
"""Contrastive loss kernel for 8 Trainium2 NeuronCores.

Math (reference):
    s = cosine similarity matrix of x [8192, 256]
    d_i = sum_j exp(s_ij * m_ij / tau)   (m zeroes the diagonal -> diag term = 1)
    v_i = s[i, i^1]                      (adjacent-row positive pairs)
    loss = mean(log d_i - v_i / tau)

Distribution: row-shard across 8 cores. Host normalizes rows of x (0.01% of
the FLOPs), transposes to xnT [256, 8192], and per core ROTATES the columns
so each core's own 1024 rows sit at columns 0..1023.  That makes the SPMD
program position-independent: the diagonal/pair blocks are always at a fixed
(compile-time) location, while row sums are invariant to column order.

Device (per core, identical program):
    - big matmul  s_tile = xnT[:, m*128:...].T @ xnT   (bf16 in, fp32 PSUM;
      fp32r measured ~5x slower on HW despite the cost model)
    - fused exp+row-sum on the scalar engine (accum_out), reading PSUM
      supertiles [128, 2048], writing bf16 exp values to SBUF
    - exp(s_ii/tau) and exp(v_i/tau) extracted from the s=0 exp tile with
      mask-multiply-reduce on the vector engine (SBUF only)
Host combines: d_i = rowsum - exp_diag + 1; loss = mean(log d - log exp_v).
Measured ~78.9us/core one-shot (incl. 4MB input DMA) via For_i repeat-diff;
PE-bound (PE-only floor 73us: 256 MMs x [213ns stream + ~53ns LDW]).

NOTE on structure: walrus codegen allows at most ONE semaphore wait per
engine instruction, so the program is arranged so every instruction depends
on at most one not-yet-observed engine (warmup touches + observer copies).
"""

import os
import sys

import numpy as np

sys.path.insert(0, "/opt/trn_rl_repo")

import concourse.bass as bass
import concourse.tile as tile
from concourse import mybir
from concourse.bass_utils import run_bass_kernel_spmd

import os as _os_early
TAU = 0.1
N = 8192
D = 256
P = 128
NCORES = 8
ROWS_PER_CORE = N // NCORES          # 1024
M_TILES = ROWS_PER_CORE // P         # 8
SUPER = int(_os_early.environ.get("KERNEL_SUPER", "2048"))  # ACT supertile width
S_TILES = N // SUPER
SUB = SUPER // 512                   # matmuls of N=512 per supertile
CHUNK = 2048                         # input DMA chunk width
C_TILES = N // CHUNK
FP32 = mybir.dt.float32
FP32R = mybir.dt.float32r
# matmul input dtype: "fp32r" or "bf16"
import os as _os
MM_DT = _os.environ.get("KERNEL_MM_DT", "bf16")
EO_DT = _os.environ.get("KERNEL_EO_DT", "bf16")   # exp-output dtype knob

_CACHE = {}


def build_nc(repeat=1):
    mmdt = FP32R if MM_DT == "fp32r" else mybir.dt.bfloat16
    xtdt = FP32 if MM_DT == "fp32r" else mybir.dt.bfloat16
    nc = bass.Bass(trn_type="TRN2")
    xt_d = nc.declare_dram_parameter("xt", [2, P, N], xtdt, isOutput=False)
    eodt_d = FP32 if EO_DT == "fp32" else mybir.dt.bfloat16
    eye_d = nc.declare_dram_parameter("eye", [P, P], eodt_d, isOutput=False)
    pm_d = nc.declare_dram_parameter("pm", [P, P], eodt_d, isOutput=False)
    acc_d = nc.declare_dram_parameter("acc", [P, M_TILES * S_TILES], FP32, isOutput=True)
    dv_d = nc.declare_dram_parameter("dv", [P, 2 * M_TILES], FP32, isOutput=True)

    with tile.TileContext(nc) as tc:
        with (
            tc.tile_pool(name="big", bufs=2) as big,
            tc.tile_pool(name="small", bufs=1) as small,
            tc.tile_pool(name="scratch", bufs=4) as sc,
            tc.tile_pool(name="psum", bufs=int(_os.environ.get("KERNEL_PSUM_BUFS", "2")), space="PSUM") as pp,
        ):
            eodt = FP32 if EO_DT == "fp32" else mybir.dt.bfloat16
            eye = small.tile([P, P], eodt, tag="eye")
            pm = small.tile([P, P], eodt, tag="pm")
            acc_sb = small.tile([P, M_TILES * S_TILES], FP32, tag="accsb")
            dv_sb = small.tile([P, 2 * M_TILES], FP32, tag="dvsb")

            nc.sync.dma_start(out=eye, in_=eye_d[:, :])
            nc.sync.dma_start(out=pm, in_=pm_d[:, :])
            # Warmup: make DVE/ACT observe the mask DMAs (and load the Exp
            # table) before the main loop, so steady-state instructions carry
            # a single sem wait (codegen limit) and the ~2.7us ACT table load
            # happens off the critical path.
            warm_v = small.tile([P, 1], FP32, tag="warm_v")
            warm_v2 = small.tile([P, 1], FP32, tag="warm_v2")
            warm_a = small.tile([P, P], FP32, tag="warm_a")
            warm_s = small.tile([P, 1], FP32, tag="warm_s")
            nc.vector.reduce_sum(warm_v, eye, axis=mybir.AxisListType.X)
            nc.vector.reduce_sum(warm_v2, pm, axis=mybir.AxisListType.X)
            nc.scalar.activation(out=warm_a, in_=pm,
                                 func=mybir.ActivationFunctionType.Exp,
                                 scale=1.0, accum_out=warm_s)

            import contextlib
            loop_ctx = (tc.For_i(0, repeat, 1)
                        if repeat > 1 else contextlib.nullcontext())
            with loop_ctx:
                _compute_body(nc, tc, sc, pp, small, big, xt_d, mmdt,
                              eye, pm, acc_sb, dv_sb)

            if _os.environ.get("KERNEL_PE_ONLY", "0") == "1":
                nc.vector.memset(acc_sb, 0.0)
                nc.vector.memset(dv_sb, 0.0)
            nc.sync.dma_start(out=acc_d[:, :], in_=acc_sb)
            nc.sync.dma_start(out=dv_d[:, :], in_=dv_sb)
    _split_multi_waits(nc)
    return nc


def _compute_body(nc, tc, sc, pp, small, big, xt_d, mmdt,
                  eye, pm, acc_sb, dv_sb):
    if _os.environ.get("KERNEL_NULL", "0") == "1":
        nc.vector.memset(acc_sb, 0.0)
        nc.vector.memset(dv_sb, 0.0)
        return
    # x tiles live inside the (bench) loop so input DMA pipelines with the
    # previous iteration's compute; in the one-shot kernel this is just the
    # chunked load.
    xt0 = big.tile([P, N], mmdt, tag="xt0")  # d = 0..127   (k half 0)
    xt1 = big.tile([P, N], mmdt, tag="xt1")  # d = 128..255 (k half 1)
    headopt = _os.environ.get("KERNEL_HEADOPT", "1") == "1"
    if _os.environ.get("KERNEL_CHUNK_DMA", "1") == "1":
        if headopt:
            # split the first chunk pair into 512-wide pieces, k0/k1
            # interleaved, so the first matmul group can start ~1.5us in
            for p_ in range(CHUNK // 512):
                cs = slice(p_ * 512, (p_ + 1) * 512)
                nc.sync.dma_start(out=xt0[:, cs], in_=xt_d[0, :, cs].bitcast(mmdt))
                nc.sync.dma_start(out=xt1[:, cs], in_=xt_d[1, :, cs].bitcast(mmdt))
            first_c = 1
        else:
            first_c = 0
        for c_ in range(first_c, C_TILES):
            cs = slice(c_ * CHUNK, (c_ + 1) * CHUNK)
            nc.sync.dma_start(out=xt0[:, cs], in_=xt_d[0, :, cs].bitcast(mmdt))
            nc.sync.dma_start(out=xt1[:, cs], in_=xt_d[1, :, cs].bitcast(mmdt))
    else:
        nc.sync.dma_start(out=xt0, in_=xt_d[0].bitcast(mmdt))
        nc.sync.dma_start(out=xt1, in_=xt_d[1].bitcast(mmdt))
    if MM_DT == "bf16" and _os.environ.get("KERNEL_HEADOPT", "1") == "1":
        ps_warm = pp.tile([P, SUPER], FP32, tag="super")
        for _w in range(12):
            nc.tensor.matmul(ps_warm[:, 0:P], eye, eye, start=True, stop=True)
    for s in range(S_TILES):
        for m in range(M_TILES):
            lhs0 = xt0[:, m * P:(m + 1) * P]
            lhs1 = xt1[:, m * P:(m + 1) * P]
            if (m == 0 and MM_DT == "bf16" and (s * SUPER) % CHUNK == 0
                    and _os.environ.get("KERNEL_CHUNK_DMA", "1") == "1"):
                # dummy weight loads absorb the chunk-DMA waits on PE
                nc.tensor.ldweights(xt0[:, s * SUPER:s * SUPER + P])
                nc.tensor.ldweights(xt1[:, s * SUPER:s * SUPER + P])
            ps = pp.tile([P, SUPER], FP32, tag="super")
            if _os.environ.get("KERNEL_K_OUTER", "0") == "1":
                # one weight load serves 4 column slices
                for k, (lhs, xt) in enumerate(((lhs0, xt0), (lhs1, xt1))):
                    for sub in range(SUB):
                        cols = slice(s * SUPER + sub * 512,
                                     s * SUPER + (sub + 1) * 512)
                        nc.tensor.matmul(ps[:, sub * 512:(sub + 1) * 512],
                                         lhs, xt[:, cols],
                                         start=(k == 0), stop=(k == 1))
            else:
                for sub in range(SUB):
                    cols = slice(s * SUPER + sub * 512, s * SUPER + (sub + 1) * 512)
                    pslice = ps[:, sub * 512:(sub + 1) * 512]
                    nc.tensor.matmul(pslice, lhs0, xt0[:, cols],
                                     start=True, stop=False)
                    nc.tensor.matmul(pslice, lhs1, xt1[:, cols],
                                     start=False, stop=True)
            if _os.environ.get("KERNEL_PE_ONLY", "0") == "1":
                continue
            # exp + fused row-sum; s=0 exp tiles keep their own slots
            # because DVE reads them (diag/pair extraction).
            eo = sc.tile([P, SUPER], FP32 if EO_DT == "fp32" else mybir.dt.bfloat16,
                         tag="expout0" if s == 0 else "expout")
            nc.scalar.activation(
                out=eo, in_=ps, func=mybir.ActivationFunctionType.Exp,
                scale=1.0 / TAU,
                accum_out=acc_sb[:, m * S_TILES + s:m * S_TILES + s + 1])
            if s == 0:
                gblk = eo[:, m * P:(m + 1) * P]
                tmp = sc.tile([P, P], FP32, tag="gtmp")
                nc.vector.tensor_tensor(
                    out=tmp, in0=gblk, in1=eye, op=mybir.AluOpType.mult)
                nc.vector.reduce_sum(
                    dv_sb[:, m:m + 1], tmp, axis=mybir.AxisListType.X)
                tmp2 = sc.tile([P, P], FP32, tag="gtmp")
                nc.vector.tensor_tensor(
                    out=tmp2, in0=gblk, in1=pm, op=mybir.AluOpType.mult)
                nc.vector.reduce_sum(
                    dv_sb[:, M_TILES + m:M_TILES + m + 1], tmp2,
                    axis=mybir.AxisListType.X)
                # observer: let ACT see the DVE sem so the next
                # s=0 exp's buffer WAR needs no extra wait
                obs = small.tile([P, 1], FP32, tag=f"obs{m}")
                nc.scalar.copy(out=obs,
                               in_=dv_sb[:, M_TILES + m:M_TILES + m + 1])


def _split_multi_waits(nc):
    """walrus codegen accepts at most ONE semaphore wait per engine
    instruction; Tile's wait assignment can bake in several.  Hoist all but
    the last wait of each engine instruction into standalone
    InstEventSemaphore sequencer ops right before it (the same mechanism
    barriers use) — semantics are identical, the engine blocks on the waits
    in order."""
    n_split = 0
    for blk in nc.m.functions[0].blocks:
        new_insts = []
        for inst in blk.instructions:
            si = inst.sync_info
            tname = type(inst).__name__
            if si is not None and len(si.on_wait) > 1 and tname != "InstEventSemaphore":
                waits = list(si.on_wait)
                for j, w in enumerate(waits[:-1]):
                    es = mybir.InstEventSemaphore(
                        name=f"W-split-{inst.name}-{j}")
                    es.engine = inst.engine
                    es.sync_info = mybir.SyncInfo(on_wait=[w], on_update=[])
                    new_insts.append(es)
                    nc.register_instruction(es)
                    n_split += 1
                inst.sync_info = mybir.SyncInfo(
                    on_wait=[waits[-1]], on_update=list(si.on_update))
            new_insts.append(inst)
        blk.instructions[:] = new_insts
    return n_split


def _masks():
    if EO_DT == "fp32":
        mdt = np.float32
    else:
        import ml_dtypes
        mdt = ml_dtypes.bfloat16
    eye = np.eye(P, dtype=mdt)
    pm = np.zeros((P, P), dtype=mdt)
    idx = np.arange(P)
    pm[idx, idx ^ 1] = mdt(1.0)
    return eye, pm


def _prepare_inputs(x):
    x = np.ascontiguousarray(np.asarray(x, dtype=np.float32))
    inv = 1.0 / np.sqrt((x * x).sum(axis=1))
    xn = x * inv[:, None].astype(np.float32)
    if MM_DT == "bf16":
        import ml_dtypes
        xnT = np.ascontiguousarray(xn.T.astype(ml_dtypes.bfloat16))
    else:
        xnT = np.ascontiguousarray(xn.T.astype(np.float32))  # [256, 8192]
    eye, pm = _masks()
    in_maps = []
    for c in range(NCORES):
        rolled = np.roll(xnT, -c * ROWS_PER_CORE, axis=1)
        xt = np.ascontiguousarray(rolled.reshape(2, P, N))
        in_maps.append({"xt": xt, "eye": eye, "pm": pm})
    return in_maps


def _combine(results):
    total = 0.0
    for c in range(NCORES):
        acc = np.asarray(results[c]["acc"], dtype=np.float64)   # [128, 32]
        dv = np.asarray(results[c]["dv"], dtype=np.float64)     # [128, 16]
        rowsum = acc.reshape(P, M_TILES, S_TILES).sum(axis=2)   # [p, m]
        diag_exp = dv[:, :M_TILES]                              # exp(s_ii/tau)
        v_exp = dv[:, M_TILES:]                                 # exp(v_i/tau)
        d = rowsum - diag_exp + 1.0
        total += (np.log(d) - np.log(v_exp)).sum()
    return np.float32(total / N)


def kernel(x, repeat=None):
    if repeat is None:
        repeat = int(os.environ.get("KERNEL_REPEAT", "1"))
    key = f"nc{repeat}"
    if key not in _CACHE:
        _CACHE[key] = build_nc(repeat)
    nc = _CACHE[key]
    in_maps = _prepare_inputs(x)
    trace = bool(int(os.environ.get("KERNEL_TRACE", "0")))
    res = run_bass_kernel_spmd(nc, in_maps, list(range(NCORES)), trace=trace)
    _CACHE["last_results"] = res
    return _combine(res.results)



# revision 7
# speedup vs baseline: 1.5198x; 1.5198x over previous
"""Contrastive loss kernel for 8 Trainium2 NeuronCores — symmetric + fp8.

Math (reference):
    s = cosine similarity matrix of x [8192, 256]
    d_i = sum_j exp(s_ij * m_ij / tau)   (m zeroes the diagonal -> diag term = 1)
    v_i = s[i, i^1]                      (adjacent-row positive pairs)
    loss = mean(log d_i - v_i / tau)

Key ideas over the v1 kernel (78.9us, full [1024, 8192] slice per core):
  1. SYMMETRY: s is symmetric, so exp(s) is too.  In 128-row block units
     (64x64 block grid) each core computes, for each of its 8 block rows m,
     only the 33 blocks at cyclic distance d = 0..32 (columns m..m+32 in its
     ROLLED coordinates).  Each off-diagonal exp block is then used twice:
     its ACT accum row-sum covers (row block m, d=0..32), and its COLUMN sum
     (accumulated on DVE, reduced with a ones-matmul on PE) covers the
     mirrored blocks at d = -31..-1.  Every row's 64 column blocks are
     covered exactly once; the d=32 ring is computed from both sides with
     row-sums only.  => PE matmul + ACT exp work drop ~2x.
  2. FP8 (e4m3) matmul with perf_mode=DoubleRow: K=256 contraction in ONE
     pass (weights [128, 2, 128], moving [128, 2, 512]).  Host-simulated
     end-to-end rel-err of the full scheme: 5.7e-5 (gate 2e-3).
  3. The host normalizes rows, quantizes to fp8, and rotates columns per
     core exactly as v1 did (position-independent SPMD program).

Per-core engine budget (predicted): ACT ~30.5us (bottleneck: 264 exp blocks
@ 1 elem/lane/cycle @ 1.2GHz + 172c/instr), PE ~20us (DoubleRow MMs + ones
column reduce), DVE ~24us (colsum accumulate bf16 2x + extractions).

NOTE on structure: walrus codegen allows at most ONE semaphore wait per
engine instruction; _split_multi_waits() hoists extras into standalone
InstEventSemaphore ops.
"""

import os
import sys

import numpy as np

sys.path.insert(0, "/opt/trn_rl_repo")

import concourse.bass as bass
import concourse.tile as tile
from concourse import mybir
from concourse.bass_utils import run_bass_kernel_spmd

TAU = 0.1
N = 8192
D = 256
P = 128
NCORES = 8
NB = N // P                      # 64 block rows globally
M_TILES = 8                      # block rows per core
NCOL = 40                        # column blocks needed per core (m..m+32, m<=7)
W = NCOL * P                     # 5120 streamed columns per core
SUPER = 2048                     # PSUM supertile width (16 blocks)
JCS = 38                         # colsum j-blocks: j = 1..38
CSW = JCS * P                    # 4864 colsum accumulator width
FP32 = mybir.dt.float32
BF16 = mybir.dt.bfloat16
FP8 = mybir.dt.float8e4

_CACHE = {}


def build_nc(repeat=1):
    nc = bass.Bass(trn_type="TRN2")
    xt_d = nc.declare_dram_parameter("xt", [P, 2, W], FP8, isOutput=False)
    eye_d = nc.declare_dram_parameter("eye", [P, P], BF16, isOutput=False)
    pm_d = nc.declare_dram_parameter("pm", [P, P], BF16, isOutput=False)
    # acc: cols 0..15 = ACT accum (m,S1),(m,S2); dv: 0..7 d32 rowsum,
    # 8..15 diag exp, 16..23 pair exp
    acc_d = nc.declare_dram_parameter("acc", [P, 2 * M_TILES], FP32, isOutput=True)
    dv_d = nc.declare_dram_parameter("dv", [P, 3 * M_TILES], FP32, isOutput=True)
    cs_d = nc.declare_dram_parameter("cs", [P, JCS], FP32, isOutput=True)

    with tile.TileContext(nc) as tc:
        with (
            tc.tile_pool(name="big", bufs=2) as big,
            tc.tile_pool(name="small", bufs=1) as small,
            tc.tile_pool(name="scratch", bufs=4) as sc,
            tc.tile_pool(name="psum", bufs=2, space="PSUM") as pp,
        ):
            eye = small.tile([P, P], BF16, tag="eye")
            pm = small.tile([P, P], BF16, tag="pm")
            ones = small.tile([P, P], BF16, tag="ones")
            acc_sb = small.tile([P, 2 * M_TILES], FP32, tag="accsb")
            dv_sb = small.tile([P, 3 * M_TILES], FP32, tag="dvsb")

            nc.sync.dma_start(out=eye, in_=eye_d[:, :])
            nc.sync.dma_start(out=pm, in_=pm_d[:, :])
            nc.vector.memset(ones, 1.0)
            # Warmup: make DVE/ACT observe the mask DMAs (and load the Exp
            # table off the critical path).
            warm_v = small.tile([P, 1], FP32, tag="warm_v")
            warm_v2 = small.tile([P, 1], FP32, tag="warm_v2")
            warm_a = small.tile([P, P], FP32, tag="warm_a")
            warm_s = small.tile([P, 1], FP32, tag="warm_s")
            nc.vector.reduce_sum(warm_v, eye, axis=mybir.AxisListType.X)
            nc.vector.reduce_sum(warm_v2, pm, axis=mybir.AxisListType.X)
            nc.scalar.activation(out=warm_a, in_=pm,
                                 func=mybir.ActivationFunctionType.Exp,
                                 scale=1.0, accum_out=warm_s)

            import contextlib
            loop_ctx = (tc.For_i(0, repeat, 1)
                        if repeat > 1 else contextlib.nullcontext())
            with loop_ctx:
                _compute_body(nc, tc, sc, pp, small, big, xt_d, cs_d,
                              eye, pm, ones, acc_sb, dv_sb)

            if os.environ.get("KERNEL_PE_ONLY", "0") == "1":
                nc.vector.memset(acc_sb, 0.0)
                nc.vector.memset(dv_sb, 0.0)
            nc.sync.dma_start(out=acc_d[:, :], in_=acc_sb)
            nc.sync.dma_start(out=dv_d[:, :], in_=dv_sb)
    _split_multi_waits(nc)
    return nc


def _compute_body(nc, tc, sc, pp, small, big, xt_d, cs_d,
                  eye, pm, ones, acc_sb, dv_sb):
    pe_only = os.environ.get("KERNEL_PE_ONLY", "0") == "1"
    if os.environ.get("KERNEL_NULL", "0") == "1":
        nc.vector.memset(acc_sb, 0.0)
        nc.vector.memset(dv_sb, 0.0)
        zz = big.tile([P, JCS], FP32, tag="zz")
        nc.vector.memset(zz, 0.0)
        nc.sync.dma_start(out=cs_d[:, :], in_=zz)
        return
    DR = mybir.MatmulPerfMode.DoubleRow
    EXP = mybir.ActivationFunctionType.Exp

    xt = big.tile([P, 2, W], FP8, tag="xt")       # fp8 rolled columns
    A = big.tile([P, CSW], BF16, tag="A")         # colsum accumulator j=1..38
    nc.vector.memset(A, 0.0)

    # chunked input DMA; first chunk split fine so m=0 can start early
    CH = 1024
    for p_ in range(4):
        cslc = slice(p_ * 512, (p_ + 1) * 512)
        nc.sync.dma_start(out=xt[:, :, cslc], in_=xt_d[:, :, cslc])
    for c_ in range(2, W // CH):
        cslc = slice(c_ * CH, (c_ + 1) * CH)
        nc.sync.dma_start(out=xt[:, :, cslc], in_=xt_d[:, :, cslc])

    # PE warmup on masks (also absorbs DMA semaphore observations)
    ps_warm = pp.tile([P, SUPER], FP32, tag="super")
    for _w in range(12):
        nc.tensor.matmul(ps_warm[:, 0:P], eye, eye, start=True, stop=True)

    def mm_supertile(m, s_idx):
        """matmul supertile s_idx (0/1) of block row m -> psum tile."""
        ps = pp.tile([P, SUPER], FP32, tag="super")
        lhs = xt[:, :, m * P:(m + 1) * P]
        base = m * P + s_idx * SUPER
        if s_idx == 0:
            # dummy weight loads absorb chunk-DMA waits on PE
            nc.tensor.ldweights(xt[:, 0, base:base + P])
        for k in range(SUPER // 512):
            cols = slice(base + k * 512, base + (k + 1) * 512)
            nc.tensor.matmul(ps[:, k * 512:(k + 1) * 512], lhs,
                             xt[:, :, cols], start=True, stop=True,
                             perf_mode=DR)
        return ps

    def act_exp(ps, m, col, width=SUPER):
        eo = sc.tile([P, SUPER], BF16, tag="eo")
        nc.scalar.activation(out=eo[:, :width], in_=ps[:, :width], func=EXP,
                             scale=1.0 / TAU,
                             accum_out=acc_sb[:, col:col + 1])
        return eo

    for m in range(M_TILES):
        ps1 = mm_supertile(m, 0)
        ps2 = mm_supertile(m, 1)
        if m == 4:
            # d32 strip pass mid-kernel (columns all arrived by now):
            # one [128,128] block per m' at cyclic distance 32
            ps32 = pp.tile([P, SUPER], FP32, tag="super")
            for m_ in range(M_TILES):
                lhs = xt[:, :, m_ * P:(m_ + 1) * P]
                cols = slice((m_ + 32) * P, (m_ + 33) * P)
                nc.tensor.matmul(ps32[:, m_ * P:(m_ + 1) * P], lhs,
                                 xt[:, :, cols], start=True, stop=True,
                                 perf_mode=DR)
        if pe_only:
            continue
        eo1 = act_exp(ps1, m, 2 * m)
        # diag + pair extraction from the d=0 block (mask-mult + reduce)
        g1 = sc.tile([P, P], BF16, tag="gtmp")
        nc.vector.tensor_tensor(out=g1, in0=eo1[:, 0:P], in1=eye,
                                op=mybir.AluOpType.mult)
        nc.vector.reduce_sum(dv_sb[:, M_TILES + m:M_TILES + m + 1], g1,
                             axis=mybir.AxisListType.X)
        g2 = sc.tile([P, P], BF16, tag="gtmp")
        nc.vector.tensor_tensor(out=g2, in0=eo1[:, 0:P], in1=pm,
                                op=mybir.AluOpType.mult)
        nc.vector.reduce_sum(dv_sb[:, 2 * M_TILES + m:2 * M_TILES + m + 1],
                             g2, axis=mybir.AxisListType.X)
        # colsum accumulate: A cols are j-1 blocks; m covers j=m+1..m+31
        nc.vector.tensor_tensor(
            out=A[:, m * P:m * P + 15 * P], in0=A[:, m * P:m * P + 15 * P],
            in1=eo1[:, P:SUPER], op=mybir.AluOpType.add)
        eo2 = act_exp(ps2, m, 2 * m + 1)
        nc.vector.tensor_tensor(
            out=A[:, m * P + 15 * P:m * P + 31 * P],
            in0=A[:, m * P + 15 * P:m * P + 31 * P],
            in1=eo2[:, 0:SUPER], op=mybir.AluOpType.add)
        if m == 4:
            eo32 = sc.tile([P, M_TILES * P], BF16, tag="eo32")
            nc.scalar.activation(out=eo32, in_=ps32[:, :M_TILES * P],
                                 func=EXP, scale=1.0 / TAU)
            nc.vector.reduce_sum(
                dv_sb[:, 0:M_TILES],
                eo32.rearrange("p (g c) -> p g c", g=M_TILES),
                axis=mybir.AxisListType.X)

    if pe_only:
        return
    # final column reduce of A: per j-block, A_block.T @ ones_col gives the
    # partition-sum as a [128, 1] psum column (PE is the only cross-partition
    # reducer; DMA cannot read PSUM so keep the result compact)
    psC = pp.tile([P, SUPER], FP32, tag="super")
    for jj in range(JCS):
        nc.tensor.matmul(psC[:, jj:jj + 1], A[:, jj * P:(jj + 1) * P],
                         ones[:, 0:1], start=True, stop=True)
    cs_sb = small.tile([P, JCS], FP32, tag="cssb")
    nc.vector.tensor_copy(out=cs_sb, in_=psC[:, 0:JCS])
    nc.sync.dma_start(out=cs_d[:, :], in_=cs_sb)


def _split_multi_waits(nc):
    """walrus codegen accepts at most ONE semaphore wait per engine
    instruction; hoist all but the last wait into standalone
    InstEventSemaphore sequencer ops right before it."""
    n_split = 0
    for blk in nc.m.functions[0].blocks:
        new_insts = []
        for inst in blk.instructions:
            si = inst.sync_info
            tname = type(inst).__name__
            if si is not None and len(si.on_wait) > 1 and tname != "InstEventSemaphore":
                waits = list(si.on_wait)
                for j, w in enumerate(waits[:-1]):
                    es = mybir.InstEventSemaphore(
                        name=f"W-split-{inst.name}-{j}")
                    es.engine = inst.engine
                    es.sync_info = mybir.SyncInfo(on_wait=[w], on_update=[])
                    new_insts.append(es)
                    nc.register_instruction(es)
                    n_split += 1
                inst.sync_info = mybir.SyncInfo(
                    on_wait=[waits[-1]], on_update=list(si.on_update))
            new_insts.append(inst)
        blk.instructions[:] = new_insts
    return n_split


def _masks():
    import ml_dtypes
    mdt = ml_dtypes.bfloat16
    eye = np.eye(P, dtype=mdt)
    pm = np.zeros((P, P), dtype=mdt)
    idx = np.arange(P)
    pm[idx, idx ^ 1] = mdt(1.0)
    return eye, pm


def _prepare_inputs(x):
    import ml_dtypes
    x = np.ascontiguousarray(np.asarray(x, dtype=np.float32))
    inv = 1.0 / np.sqrt((x * x).sum(axis=1))
    xn = x * inv[:, None].astype(np.float32)
    q = xn.astype(ml_dtypes.float8_e4m3)             # [8192, 256] fp8
    eye, pm = _masks()
    in_maps = []
    for c in range(NCORES):
        rolled = np.roll(q, -c * (N // NCORES), axis=0)   # rolled rows
        # xt[p, ko, col] = rolled[col, ko*128 + p]; only first W columns
        xt = np.ascontiguousarray(
            rolled[:W].T.reshape(2, P, W).transpose(1, 0, 2))
        in_maps.append({"xt": xt, "eye": eye, "pm": pm})
    return in_maps


def _combine(results):
    Dsum = np.zeros(N, dtype=np.float64)
    DIAG = np.zeros(N, dtype=np.float64)
    VEXP = np.zeros(N, dtype=np.float64)
    p_ = np.arange(P)
    for c in range(NCORES):
        acc = np.asarray(results[c]["acc"], dtype=np.float64)  # [128, 16]
        dv = np.asarray(results[c]["dv"], dtype=np.float64)    # [128, 24]
        cs = np.asarray(results[c]["cs"], dtype=np.float64)   # [128, 38]
        for m in range(M_TILES):
            g = ((8 * c + m) % NB) * P + p_
            Dsum[g] += acc[:, 2 * m] + acc[:, 2 * m + 1] + dv[:, m]
            DIAG[g] = dv[:, M_TILES + m]
            VEXP[g] = dv[:, 2 * M_TILES + m]
        for jj in range(1, JCS + 1):
            gb = ((8 * c + jj) % NB) * P
            Dsum[gb:gb + P] += cs[:, jj - 1]
    d = Dsum - DIAG + 1.0
    loss = np.mean(np.log(d) - np.log(VEXP))
    return np.float32(loss)


def kernel(x, repeat=None):
    if repeat is None:
        repeat = int(os.environ.get("KERNEL_REPEAT", "1"))
    key = f"nc{repeat}"
    if key not in _CACHE:
        _CACHE[key] = build_nc(repeat)
    nc = _CACHE[key]
    in_maps = _prepare_inputs(x)
    trace = bool(int(os.environ.get("KERNEL_TRACE", "0")))
    res = run_bass_kernel_spmd(nc, in_maps, list(range(NCORES)), trace=trace)
    _CACHE["last_results"] = res
    return _combine(res.results)


# revision 10
# speedup vs baseline: 3.9943x; 2.6282x over previous
"""Contrastive loss kernel for 8 Trainium2 NeuronCores — symmetric + fp8.

Math (reference):
    s = cosine similarity matrix of x [8192, 256]
    d_i = sum_j exp(s_ij * m_ij / tau)   (m zeroes the diagonal -> diag term = 1)
    v_i = s[i, i^1]                      (adjacent-row positive pairs)
    loss = mean(log d_i - v_i / tau)

Key ideas over the v1 kernel (78.9us, full [1024, 8192] slice per core):
  1. SYMMETRY: s is symmetric, so exp(s) is too.  In 128-row block units
     (64x64 block grid) each core computes, for each of its 8 block rows m,
     only the 33 blocks at cyclic distance d = 0..32 (columns m..m+32 in its
     ROLLED coordinates).  Each off-diagonal exp block is then used twice:
     its ACT accum row-sum covers (row block m, d=0..32), and its COLUMN sum
     (accumulated on DVE, reduced with a ones-matmul on PE) covers the
     mirrored blocks at d = -31..-1.  Every row's 64 column blocks are
     covered exactly once; the d=32 ring is computed from both sides with
     row-sums only.  => PE matmul + ACT exp work drop ~2x.
  2. FP8 (e4m3) matmul with perf_mode=DoubleRow: K=256 contraction in ONE
     pass (weights [128, 2, 128], moving [128, 2, 512]).  Host-simulated
     end-to-end rel-err of the full scheme: 5.7e-5 (gate 2e-3).
  3. The host normalizes rows, quantizes to fp8, and rotates columns per
     core exactly as v1 did (position-independent SPMD program).

Per-core engine budget (predicted): ACT ~30.5us (bottleneck: 264 exp blocks
@ 1 elem/lane/cycle @ 1.2GHz + 172c/instr), PE ~20us (DoubleRow MMs + ones
column reduce), DVE ~24us (colsum accumulate bf16 2x + extractions).

NOTE on structure: walrus codegen allows at most ONE semaphore wait per
engine instruction; _split_multi_waits() hoists extras into standalone
InstEventSemaphore ops.
"""

import os
import sys

import numpy as np

sys.path.insert(0, "/opt/trn_rl_repo")

import concourse.bass as bass
import concourse.tile as tile
from concourse import mybir
from concourse.bass_utils import run_bass_kernel_spmd

TAU = 0.1
N = 8192
D = 256
P = 128
NCORES = 8
NB = N // P                      # 64 block rows globally
M_TILES = 8                      # block rows per core
NCOL = 40                        # column blocks needed per core (m..m+32, m<=7)
W = NCOL * P                     # 5120 streamed columns per core
SUPER = 2048                     # PSUM supertile width (16 blocks)
JCS = 38                         # colsum j-blocks: j = 1..38
CSW = JCS * P                    # 4864 colsum accumulator width
FP32 = mybir.dt.float32
BF16 = mybir.dt.bfloat16
FP8 = mybir.dt.float8e4

_CACHE = {}


def build_nc(repeat=1):
    nc = bass.Bass(trn_type="TRN2")
    xt_d = nc.declare_dram_parameter("xt", [P, 2, W], FP8, isOutput=False)
    eye_d = nc.declare_dram_parameter("eye", [P, P], BF16, isOutput=False)
    pm_d = nc.declare_dram_parameter("pm", [P, P], BF16, isOutput=False)
    # acc: cols 0..15 = ACT accum (m,S1),(m,S2); dv: 0..7 d32 rowsum,
    # 8..15 diag exp, 16..23 pair exp
    acc_d = nc.declare_dram_parameter("acc", [P, 2 * M_TILES], FP32, isOutput=True)
    dv_d = nc.declare_dram_parameter("dv", [P, 3 * M_TILES], FP32, isOutput=True)
    cs_d = nc.declare_dram_parameter("cs", [P, JCS], FP32, isOutput=True)

    with tile.TileContext(nc) as tc:
        with (
            tc.tile_pool(name="big", bufs=2) as big,
            tc.tile_pool(name="small", bufs=1) as small,
            tc.tile_pool(name="scratch", bufs=4) as sc,
            tc.tile_pool(name="psum", bufs=2, space="PSUM") as pp,
        ):
            eye = small.tile([P, P], BF16, tag="eye")
            pm = small.tile([P, P], BF16, tag="pm")
            ones = small.tile([P, P], BF16, tag="ones")
            acc_sb = small.tile([P, 2 * M_TILES], FP32, tag="accsb")
            dv_sb = small.tile([P, 3 * M_TILES], FP32, tag="dvsb")

            nc.sync.dma_start(out=eye, in_=eye_d[:, :])
            nc.sync.dma_start(out=pm, in_=pm_d[:, :])
            nc.vector.memset(ones, 1.0)
            # Warmup: make DVE/ACT observe the mask DMAs (and load the Exp
            # table off the critical path).
            warm_v = small.tile([P, 1], FP32, tag="warm_v")
            warm_v2 = small.tile([P, 1], FP32, tag="warm_v2")
            warm_a = small.tile([P, P], FP32, tag="warm_a")
            warm_s = small.tile([P, 1], FP32, tag="warm_s")
            nc.vector.reduce_sum(warm_v, eye, axis=mybir.AxisListType.X)
            nc.vector.reduce_sum(warm_v2, pm, axis=mybir.AxisListType.X)
            nc.scalar.activation(out=warm_a, in_=pm,
                                 func=mybir.ActivationFunctionType.Exp,
                                 scale=1.0, accum_out=warm_s)

            import contextlib
            loop_ctx = (tc.For_i(0, repeat, 1)
                        if repeat > 1 else contextlib.nullcontext())
            with loop_ctx:
                _compute_body(nc, tc, sc, pp, small, big, xt_d, cs_d,
                              eye, pm, ones, acc_sb, dv_sb)

            if os.environ.get("KERNEL_PE_ONLY", "0") == "1":
                nc.vector.memset(acc_sb, 0.0)
                nc.vector.memset(dv_sb, 0.0)
            nc.sync.dma_start(out=acc_d[:, :], in_=acc_sb)
            nc.sync.dma_start(out=dv_d[:, :], in_=dv_sb)
    _split_multi_waits(nc)
    return nc


def _compute_body(nc, tc, sc, pp, small, big, xt_d, cs_d,
                  eye, pm, ones, acc_sb, dv_sb):
    pe_only = os.environ.get("KERNEL_PE_ONLY", "0") == "1"
    no_dve = os.environ.get("KERNEL_NO_DVE", "0") == "1"
    if os.environ.get("KERNEL_NULL", "0") == "1":
        nc.vector.memset(acc_sb, 0.0)
        nc.vector.memset(dv_sb, 0.0)
        zz = big.tile([P, JCS], FP32, tag="zz")
        nc.vector.memset(zz, 0.0)
        nc.sync.dma_start(out=cs_d[:, :], in_=zz)
        return
    DR = mybir.MatmulPerfMode.DoubleRow
    EXP = mybir.ActivationFunctionType.Exp

    xt = big.tile([P, 2, W], FP8, tag="xt")       # fp8 rolled columns
    A = big.tile([P, CSW], BF16, tag="A")         # colsum accumulator j=1..38
    nc.vector.memset(A, 0.0)

    # chunked input DMA; first chunk split fine so m=0 can start early
    CH = 1024
    for p_ in range(4):
        cslc = slice(p_ * 512, (p_ + 1) * 512)
        nc.sync.dma_start(out=xt[:, :, cslc], in_=xt_d[:, :, cslc])
    for c_ in range(2, W // CH):
        cslc = slice(c_ * CH, (c_ + 1) * CH)
        nc.sync.dma_start(out=xt[:, :, cslc], in_=xt_d[:, :, cslc])

    # PE warmup on masks (also absorbs DMA semaphore observations)
    ps_warm = pp.tile([P, SUPER], FP32, tag="super")
    for _w in range(12):
        nc.tensor.matmul(ps_warm[:, 0:P], eye, eye, start=True, stop=True)

    def mm_supertile(m, s_idx):
        """matmul supertile s_idx (0/1) of block row m -> psum tile."""
        ps = pp.tile([P, SUPER], FP32, tag="super")
        lhs = xt[:, :, m * P:(m + 1) * P]
        base = m * P + s_idx * SUPER
        if s_idx == 0:
            # dummy weight loads absorb chunk-DMA waits on PE
            nc.tensor.ldweights(xt[:, 0, base:base + P])
        for k in range(SUPER // 512):
            cols = slice(base + k * 512, base + (k + 1) * 512)
            nc.tensor.matmul(ps[:, k * 512:(k + 1) * 512], lhs,
                             xt[:, :, cols], start=True, stop=True,
                             perf_mode=DR)
        return ps

    def act_exp(ps, m, col, width=SUPER):
        eo = sc.tile([P, SUPER], BF16, tag="eo")
        nc.scalar.activation(out=eo[:, :width], in_=ps[:, :width], func=EXP,
                             scale=1.0 / TAU,
                             accum_out=acc_sb[:, col:col + 1])
        return eo

    for m in range(M_TILES):
        ps1 = mm_supertile(m, 0)
        ps2 = mm_supertile(m, 1)
        if m == 4:
            # d32 strip pass mid-kernel (columns all arrived by now):
            # one [128,128] block per m' at cyclic distance 32
            ps32 = pp.tile([P, SUPER], FP32, tag="super")
            for m_ in range(M_TILES):
                lhs = xt[:, :, m_ * P:(m_ + 1) * P]
                cols = slice((m_ + 32) * P, (m_ + 33) * P)
                nc.tensor.matmul(ps32[:, m_ * P:(m_ + 1) * P], lhs,
                                 xt[:, :, cols], start=True, stop=True,
                                 perf_mode=DR)
        if pe_only:
            continue
        eo1 = act_exp(ps1, m, 2 * m)
        if no_dve:
            eo2 = act_exp(ps2, m, 2 * m + 1)
            if m == 4:
                eo32 = sc.tile([P, M_TILES * P], BF16, tag="eo32")
                nc.scalar.activation(out=eo32, in_=ps32[:, :M_TILES * P],
                                     func=EXP, scale=1.0 / TAU)
            continue
        # diag + pair extraction from the d=0 block (mask-mult + reduce)
        g1 = sc.tile([P, P], BF16, tag="gtmp")
        nc.vector.tensor_tensor(out=g1, in0=eo1[:, 0:P], in1=eye,
                                op=mybir.AluOpType.mult)
        nc.vector.reduce_sum(dv_sb[:, M_TILES + m:M_TILES + m + 1], g1,
                             axis=mybir.AxisListType.X)
        g2 = sc.tile([P, P], BF16, tag="gtmp")
        nc.vector.tensor_tensor(out=g2, in0=eo1[:, 0:P], in1=pm,
                                op=mybir.AluOpType.mult)
        nc.vector.reduce_sum(dv_sb[:, 2 * M_TILES + m:2 * M_TILES + m + 1],
                             g2, axis=mybir.AxisListType.X)
        # colsum accumulate: A cols are j-1 blocks; m covers j=m+1..m+31
        nc.vector.tensor_tensor(
            out=A[:, m * P:m * P + 15 * P], in0=A[:, m * P:m * P + 15 * P],
            in1=eo1[:, P:SUPER], op=mybir.AluOpType.add)
        eo2 = act_exp(ps2, m, 2 * m + 1)
        nc.vector.tensor_tensor(
            out=A[:, m * P + 15 * P:m * P + 31 * P],
            in0=A[:, m * P + 15 * P:m * P + 31 * P],
            in1=eo2[:, 0:SUPER], op=mybir.AluOpType.add)
        if m == 4:
            eo32 = sc.tile([P, M_TILES * P], BF16, tag="eo32")
            nc.scalar.activation(out=eo32, in_=ps32[:, :M_TILES * P],
                                 func=EXP, scale=1.0 / TAU)
            nc.vector.reduce_sum(
                dv_sb[:, 0:M_TILES],
                eo32.rearrange("p (g c) -> p g c", g=M_TILES),
                axis=mybir.AxisListType.X)

    if pe_only:
        return
    if no_dve:
        nc.vector.memset(dv_sb, 0.0)
        cs0 = small.tile([P, JCS], FP32, tag="cssb")
        nc.vector.memset(cs0, 0.0)
        nc.sync.dma_start(out=cs_d[:, :], in_=cs0)
        return
    # final column reduce of A: per j-block, A_block.T @ ones_col gives the
    # partition-sum as a [128, 1] psum column (PE is the only cross-partition
    # reducer; DMA cannot read PSUM so keep the result compact)
    psC = pp.tile([P, SUPER], FP32, tag="super")
    for jj in range(JCS):
        nc.tensor.matmul(psC[:, jj:jj + 1], A[:, jj * P:(jj + 1) * P],
                         ones[:, 0:1], start=True, stop=True)
    cs_sb = small.tile([P, JCS], FP32, tag="cssb")
    nc.vector.tensor_copy(out=cs_sb, in_=psC[:, 0:JCS])
    nc.sync.dma_start(out=cs_d[:, :], in_=cs_sb)


def _split_multi_waits(nc):
    """walrus codegen accepts at most ONE semaphore wait per engine
    instruction; hoist all but the last wait into standalone
    InstEventSemaphore sequencer ops right before it."""
    n_split = 0
    for blk in nc.m.functions[0].blocks:
        new_insts = []
        for inst in blk.instructions:
            si = inst.sync_info
            tname = type(inst).__name__
            if si is not None and len(si.on_wait) > 1 and tname != "InstEventSemaphore":
                waits = list(si.on_wait)
                for j, w in enumerate(waits[:-1]):
                    es = mybir.InstEventSemaphore(
                        name=f"W-split-{inst.name}-{j}")
                    es.engine = inst.engine
                    es.sync_info = mybir.SyncInfo(on_wait=[w], on_update=[])
                    new_insts.append(es)
                    nc.register_instruction(es)
                    n_split += 1
                inst.sync_info = mybir.SyncInfo(
                    on_wait=[waits[-1]], on_update=list(si.on_update))
            new_insts.append(inst)
        blk.instructions[:] = new_insts
    return n_split


def _masks():
    import ml_dtypes
    mdt = ml_dtypes.bfloat16
    eye = np.eye(P, dtype=mdt)
    pm = np.zeros((P, P), dtype=mdt)
    idx = np.arange(P)
    pm[idx, idx ^ 1] = mdt(1.0)
    return eye, pm


def _prepare_inputs(x):
    import ml_dtypes
    x = np.ascontiguousarray(np.asarray(x, dtype=np.float32))
    inv = 1.0 / np.sqrt((x * x).sum(axis=1))
    xn = x * inv[:, None].astype(np.float32)
    q = xn.astype(ml_dtypes.float8_e4m3)             # [8192, 256] fp8
    eye, pm = _masks()
    in_maps = []
    for c in range(NCORES):
        rolled = np.roll(q, -c * (N // NCORES), axis=0)   # rolled rows
        # xt[p, ko, col] = rolled[col, ko*128 + p]; only first W columns
        xt = np.ascontiguousarray(
            rolled[:W].T.reshape(2, P, W).transpose(1, 0, 2))
        in_maps.append({"xt": xt, "eye": eye, "pm": pm})
    return in_maps


def _combine(results):
    Dsum = np.zeros(N, dtype=np.float64)
    DIAG = np.zeros(N, dtype=np.float64)
    VEXP = np.zeros(N, dtype=np.float64)
    p_ = np.arange(P)
    for c in range(NCORES):
        acc = np.asarray(results[c]["acc"], dtype=np.float64)  # [128, 16]
        dv = np.asarray(results[c]["dv"], dtype=np.float64)    # [128, 24]
        cs = np.asarray(results[c]["cs"], dtype=np.float64)   # [128, 38]
        for m in range(M_TILES):
            g = ((8 * c + m) % NB) * P + p_
            Dsum[g] += acc[:, 2 * m] + acc[:, 2 * m + 1] + dv[:, m]
            DIAG[g] = dv[:, M_TILES + m]
            VEXP[g] = dv[:, 2 * M_TILES + m]
        for jj in range(1, JCS + 1):
            gb = ((8 * c + jj) % NB) * P
            Dsum[gb:gb + P] += cs[:, jj - 1]
    d = Dsum - DIAG + 1.0
    loss = np.mean(np.log(d) - np.log(VEXP))
    return np.float32(loss)


def kernel(x, repeat=None):
    if repeat is None:
        repeat = int(os.environ.get("KERNEL_REPEAT", "1"))
    key = f"nc{repeat}"
    if key not in _CACHE:
        _CACHE[key] = build_nc(repeat)
    nc = _CACHE[key]
    in_maps = _prepare_inputs(x)
    trace = bool(int(os.environ.get("KERNEL_TRACE", "0")))
    res = run_bass_kernel_spmd(nc, in_maps, list(range(NCORES)), trace=trace)
    _CACHE["last_results"] = res
    return _combine(res.results)
